# revision 8
# baseline (speedup 1.0000x reference)
"""MultiHeadDiffAttn Trainium2 kernel (v4, descending s-loop + ping-pong S).

Sharding: 8 cores = 4-way data parallel over batch x 2-way tensor parallel
over heads (8 v-heads / 16 half-heads per core).  Each core computes its
batch's qkv projection restricted to its head group, differential attention
with per-half-head softmax, head RMS norm, and a partial output projection
(its 512 rows of w_proj).  Host sums the two partial projections per batch.

Key device-level choices:
  - all matmul operands are fp16 (fp32 streams at 1/4 rate); PSUM stays fp32.
  - S^T = k^T q per half-head contracts over only 32 dims, so it runs as
    K=32 row-tiled matmuls: the two halves of a v-head sit in different
    32-row groups of the PE array (tile_position) and stream concurrently,
    reading q/k 128-row chunks of the qkv output directly.
  - the s-loop runs DESCENDING (s=7..0): early iterations have short
    S/exp ops so the S->exp->AV pipeline ramps without serial stalls, and
    for s>=4 the S tile ping-pongs between the two halves of a [128,2048]
    PSUM tile so the next S matmul never waits for exp to finish reading.
  - exp is a single ACT op per (head, s-block) over both halves; the causal
    mask is one affine_select over both halves on the idle GpSimd engine.
  - AV accumulates U[t-block, dv|den] in PSUM (4 t-strips per bank, ones
    column 64 = softmax denominator), one s-iteration late so exps stay
    back-to-back; head 0's s=7..4 blocks are prebaked into the qkv phase.
  - the per-head epilogue is batched: one reciprocal over all 8 denominators
    per half, broadcast-AP multiplies over [128, 8, 64], and an X-axis
    tensor_reduce for the RMS statistics.
  - rstd = exp(-0.5 * ln(ssq/DV + eps)) keeps all ACT work in the single
    natural_log_exp table set, so the RMS scale for heads 0-3 runs in the
    middle of the exp stream (no table swap) and overlaps heads 4-7.
  - input DMAs are 6 ordered transfers (first-matmul dependencies first);
    y writes go out on the second HWDGE queue so the tail's transposes and
    output stores issue in parallel.
"""

import math
from contextlib import ExitStack

import numpy as np

import concourse.bass as bass
import concourse.tile as tile
from concourse import masks, mybir
from concourse.bass_utils import run_bass_kernel_spmd

# The deployed walrus rejects instructions carrying more than one sync wait
# ("Too many sync wait commands" in setupSyncWait).  Legalize at the BIR-JSON
# level: for every instruction with >1 wait, hoist the extra waits onto NoOp
# instructions inserted just before it on the same engine (engine streams are
# in-order, so semantics are identical).
_MAX_WAITS = 1


def _legalize_sync_waits(d):
    for f in d.get("functions", []):
        for bb in f.get("blocks", []):
            out = []
            for inst in bb["instructions"]:
                si = inst.get("sync_info")
                waits = (si or {}).get("on_wait") or []
                if len(waits) > _MAX_WAITS:
                    extra = waits[: len(waits) - _MAX_WAITS]
                    keep = waits[len(waits) - _MAX_WAITS :]
                    for j in range(0, len(extra), _MAX_WAITS):
                        nop = {
                            "engine": inst["engine"],
                            "ins": [],
                            "outs": [],
                            "name": f"{inst['name']}-lw{j}",
                            "opcode": "NoOp",
                            "sync_info": {
                                "on_wait": extra[j : j + _MAX_WAITS],
                                "on_update": [],
                            },
                        }
                        if "debug" in inst:
                            nop["debug"] = inst["debug"]
                        out.append(nop)
                    si["on_wait"] = keep
                out.append(inst)
            bb["instructions"] = out
    return d


_orig_to_json_bytes = bass.Bass.to_json_bytes


def _patched_to_json_bytes(self, *a, **kw):
    import json as _json

    raw = _orig_to_json_bytes(self, *a, **kw)
    return _json.dumps(_legalize_sync_waits(_json.loads(raw))).encode()


bass.Bass.to_json_bytes = _patched_to_json_bytes

F32 = mybir.dt.float32
F16 = mybir.dt.float16

B, T, C = 4, 1024, 1024
H_TOT = 16  # total v-heads
HD = 32  # half-head dim
DV = 64  # v-head dim
G = 2  # head groups (tensor parallel)
HPG = H_TOT // G  # 8 v-heads per core
COLS = 1024  # q cols + k cols per group
LAMBDA_INIT = 0.8 - 0.6 * math.exp(-0.3 * (1 - 1))  # 0.2
EPS = 1e-5
N_CORES = 8

NT = T // 128  # 8 t-tiles
NKC = C // 128  # 8 contraction chunks
N_PREBAKE = 4  # head 0 s-blocks (s=7..4) baked into the qkv phase


def _emit(ctx: ExitStack, tc: tile.TileContext, xT, w_qk, w_v, w_p, lam, y):
    nc = tc.nc
    AluOp = mybir.AluOpType
    Act = mybir.ActivationFunctionType

    const = ctx.enter_context(tc.tile_pool(name="const", bufs=1))
    ident = const.tile([128, 128], F16)
    masks.make_identity(nc, ident[:])
    lam_sb = const.tile([128, 1], F32)
    nc.sync.dma_start(out=lam_sb, in_=lam[:])
    eps_sb = const.tile([128, 1], F32)
    nc.vector.memset(eps_sb, EPS)

    big = ctx.enter_context(tc.tile_pool(name="big", bufs=1))
    qkT_sb = big.tile([128, 8, T], F16)  # row-chunks of [COLS, T]
    v_sb = big.tile([128, NT, HPG, DV + 1], F16)  # [s-chunk][head][dv | ones]
    outcat_sb = big.tile([128, NT, HPG * DV], F16)  # [t-chunk][512]
    outcatT_sb = big.tile([128, 4, T], F16)  # row-chunks of [512, T]
    wp_sb = big.tile([128, 4, C], F16)
    # combined per-head output (pre-RMS-scale), [t-part][tj][h*64+d]
    oh_sb = big.tile([128, NT, HPG * DV], F32)
    ssq_all = big.tile([128, HPG * NT], F32)  # sum(oh^2) per (h, tj)
    rstd_all = big.tile([128, HPG * NT], F32)
    lnt = big.tile([128, HPG * NT], F32)

    es_pool = ctx.enter_context(tc.tile_pool(name="es", bufs=5))

    def emit_s_mms(h, s, out_fn):
        # S^T[s-block, t] per half: K=32 row-tiled matmuls, both halves
        # concurrent in different 32-row groups of the PE array.
        # out_fn(e, c0, c1) -> PSUM AP of width c1-c0 for half e.
        t0 = 128 * s
        c_ = h // 2
        chunks = [(t0, 512), (512, 1024)] if s < 4 else [(t0, 1024)]
        for c0, c1 in chunks:
            for e in range(2):
                j = 2 * (h % 2) + e
                p0 = 32 * j
                nc.tensor.matmul(
                    out_fn(e, c0, c1),
                    qkT_sb[p0 : p0 + 32, 4 + c_, t0 : t0 + 128],
                    qkT_sb[p0 : p0 + 32, c_, c0:c1],
                    start=True,
                    stop=True,
                    tile_position=(p0, 0),
                )

    def emit_exp_mask(h, s, in_ap, es_t):
        # one exp over both halves' S rows, one causal mask for both halves
        t0 = 128 * s
        nc.scalar.activation(
            out=es_t[:, :, t0:T],
            in_=in_ap,
            func=Act.Exp,
            scale=1.0 / 32.0,
        )
        nc.gpsimd.affine_select(
            out=es_t[:, :, t0 : t0 + 128],
            in_=es_t[:, :, t0 : t0 + 128],
            pattern=[[0, 2], [1, 128]],
            compare_op=AluOp.is_ge,
            fill=0.0,
            base=0,
            channel_multiplier=-1,
        )

    def emit_rms_half(h_lo, h_hi, defer_scales):
        # rstd = (ssq/DV + eps)^-0.5 via ln+exp (stays in the
        # natural_log_exp ACT table set: no table swap mid-exp-stream)
        cols = slice(h_lo * NT, h_hi * NT)
        nc.scalar.activation(
            out=lnt[:, cols], in_=ssq_all[:, cols], func=Act.Ln,
            bias=eps_sb[:], scale=1.0 / DV,
        )
        nc.scalar.activation(
            out=rstd_all[:, cols], in_=lnt[:, cols], func=Act.Exp, scale=-0.5
        )
        rstd_r = rstd_all[:].rearrange("p (h t) -> p h t", h=HPG)
        for tj in range(NT):
            nc.vector.tensor_mul(
                outcat_sb[:, tj, h_lo * DV : h_hi * DV].rearrange(
                    "p (h d) -> p h d", h=h_hi - h_lo
                ),
                oh_sb[:, tj, h_lo * DV : h_hi * DV].rearrange(
                    "p (h d) -> p h d", h=h_hi - h_lo
                ),
                rstd_r[:, h_lo:h_hi, tj : tj + 1].broadcast_to(
                    [128, h_hi - h_lo, DV]
                ),
            )
            if not defer_scales:
                nc.sync.dma_start_transpose(
                    out=outcatT_sb[:, :, tj * 128 : (tj + 1) * 128],
                    in_=outcat_sb[:, tj, :],
                )

    # ---------------- phase 1+2: qkv projections ----------------
    prebaked = []
    with (
        tc.tile_pool(name="xw", bufs=1) as xw,
        tc.tile_pool(name="mmps", bufs=4, space="PSUM") as mmps,
    ):
        xT_sb = xw.tile([128, NKC, T], F16)
        wqk_sb = xw.tile([128, NKC, COLS], F16)
        wv_sb = xw.tile([128, NKC, 512], F16)

        xT_r = xT[:].rearrange("(c p) t -> p c t", p=128)
        wqk_r = w_qk[:].rearrange("(c p) m -> p c m", p=128)

        # ordered input DMAs: first-matmul dependencies first, few large
        # transfers (each dma_start costs ~1.1us of descriptor issue)
        nc.sync.dma_start(out=xT_sb[:, :, 0:512], in_=xT_r[:, :, 0:512])
        nc.sync.dma_start(out=wqk_sb[:, :, 0:512], in_=wqk_r[:, :, 0:512])
        nc.sync.dma_start(out=wqk_sb[:, :, 512:1024], in_=wqk_r[:, :, 512:1024])
        nc.sync.dma_start(out=xT_sb[:, :, 512:1024], in_=xT_r[:, :, 512:1024])
        nc.sync.dma_start(out=wv_sb, in_=w_v[:].rearrange("(c p) m -> p c m", p=128))
        nc.sync.dma_start(
            out=wp_sb, in_=w_p[:].rearrange("(c p) m -> p c m", p=128)
        )

        # qkT[cc-block, :] = w_qk[:, cc-block].T @ x^T
        def emit_qk_chunk(cc):
            for nh in range(2):
                ps = mmps.tile([128, 1024], F32, tag="mmps", name=f"qk{cc}{nh}")[:, 0:512]
                for kc in range(NKC):
                    nc.tensor.matmul(
                        ps,
                        wqk_sb[:, kc, cc * 128 : (cc + 1) * 128],
                        xT_sb[:, kc, nh * 512 : (nh + 1) * 512],
                        start=(kc == 0),
                        stop=(kc == NKC - 1),
                    )
                nc.vector.tensor_copy(
                    out=qkT_sb[:, cc, nh * 512 : (nh + 1) * 512], in_=ps
                )

        emit_qk_chunk(0)
        emit_qk_chunk(4)

        # prebake head 0, s=7..4: short S/exp/mask run during the rest of
        # the qkv phase so ACT warms up early.  Both halves pack into one
        # [128, 1024] PSUM tile at 512-column offsets.
        for s in range(NT - 1, NT - 1 - N_PREBAKE, -1):
            t0 = 128 * s
            w = T - t0
            ps = mmps.tile([128, 1024], F32, tag="mmps", name=f"sp{s}")
            ps_r = ps[:].rearrange("p (e w) -> p e w", e=2)
            es_t = es_pool.tile([128, 2, T], F16, tag="es", name=f"esp_{s}")
            emit_s_mms(0, s, lambda e, c0, c1: ps_r[:, e, 0 : c1 - c0])
            emit_exp_mask(0, s, ps_r[:, :, 0:w], es_t)
            prebaked.append((s, es_t))

        for cc in (1, 2, 3, 5, 6, 7):
            emit_qk_chunk(cc)

        # v[t-block, :] = x @ w_v ; scatter heads into v_sb, slot 64 = ones
        for tt in range(NT):
            ps = mmps.tile([128, 1024], F32, tag="mmps", name=f"v{tt}")[:, 0:512]
            for kc in range(NKC):
                nc.tensor.matmul(
                    ps,
                    xT_sb[:, kc, tt * 128 : (tt + 1) * 128],
                    wv_sb[:, kc, :],
                    start=(kc == 0),
                    stop=(kc == NKC - 1),
                )
            nc.vector.tensor_copy(
                out=v_sb[:, tt, :, 0:DV],
                in_=ps[:].rearrange("p (h d) -> p h d", h=HPG),
            )
            nc.vector.memset(v_sb[:, tt, :, DV : DV + 1], 1.0)

    # ---------------- phase 3: differential attention ----------------
    with (
        tc.tile_pool(name="sps", bufs=1, space="PSUM") as s_pool,
        tc.tile_pool(name="ups", bufs=1, space="PSUM") as u_pool,
        tc.tile_pool(name="comb", bufs=2) as comb,
    ):
        def emit_av(h, s, es_t):
            # U[t-block, dv|den] += expS^T[s-block, t-block].T @ v_aug[s-block]
            # descending s: bank b's first write is (s=tj, tj=4b+3), its
            # last is (s=0, tj=4b+3)
            for e in range(2):
                for tj in range(s, NT):
                    off = tj * 128
                    nc.tensor.matmul(
                        u_tiles[e][:, off : off + DV + 1],
                        es_t[:, e, off : off + 128],
                        v_sb[:, s, h, 0 : DV + 1],
                        start=(s == tj and tj % 4 == 3),
                        stop=(s == 0 and tj % 4 == 3),
                    )

        for h in range(HPG):
            # [128, 2048] f32 = 4 PSUM banks.  s<4 uses the full tile
            # (e halves at 1024-offsets); s>=4 ping-pongs between the two
            # 1024-column halves (e at 512-offsets, t stored from column 0)
            # so the next S matmul never waits on exp's PSUM read.
            s_raw = s_pool.tile([128, 2048], F32, tag="s", name=f"s_{h}")
            s_full = s_raw[:].rearrange("p (e w) -> p e w", e=2)
            s_half = s_raw[:].rearrange("p (x e w) -> p x e w", x=2, e=2)
            u_tiles = [
                u_pool.tile([128, 1024], F32, tag=f"u{e}", name=f"u{e}_{h}")
                for e in range(2)
            ]
            if h == 0:
                for k in range(N_PREBAKE - 1):
                    emit_av(h, *prebaked[k])
                prev = prebaked[N_PREBAKE - 1]
                s_start = NT - 1 - N_PREBAKE
            else:
                prev = None
                s_start = NT - 1
            for s in range(s_start, -1, -1):
                t0 = 128 * s
                w = T - t0
                es_t = es_pool.tile([128, 2, T], F16, tag="es", name=f"es_{h}_{s}")
                if s >= 4:
                    half = s % 2
                    emit_s_mms(h, s, lambda e, c0, c1: s_half[:, half, e, 0 : c1 - c0])
                    exp_in = s_half[:, half, :, 0:w]
                else:
                    emit_s_mms(h, s, lambda e, c0, c1: s_full[:, e, c0:c1])
                    exp_in = s_full[:, :, t0:T]
                if prev is not None:
                    emit_av(h, *prev)
                emit_exp_mask(h, s, exp_in, es_t)
                prev = (s, es_t)
            emit_av(h, *prev)

            # ---- batched epilogue: normalize, lambda-combine, RMS stats ----
            u_r = [u_tiles[e][:].rearrange("p (j r) -> p j r", j=NT) for e in range(2)]
            rr_ = [
                comb.tile([128, NT], F32, tag=f"r{e}", name=f"r{e}_{h}")
                for e in range(2)
            ]
            for e in range(2):
                nc.vector.reciprocal(out=rr_[e], in_=u_r[e][:, :, DV : DV + 1])
            m0 = comb.tile([128, NT, DV], F32, tag="m0", name=f"m0_{h}")
            m1 = comb.tile([128, NT, DV], F32, tag="m1", name=f"m1_{h}")
            nc.vector.scalar_tensor_tensor(
                out=m1,
                in0=u_r[1][:, :, 0:DV],
                scalar=lam_sb[:],
                in1=rr_[1][:, :, None].broadcast_to([128, NT, DV]),
                op0=AluOp.mult,
                op1=AluOp.mult,
            )
            nc.vector.tensor_mul(
                m0, u_r[0][:, :, 0:DV], rr_[0][:, :, None].broadcast_to([128, NT, DV])
            )
            oh_ap = oh_sb[:, :, h * DV : (h + 1) * DV]
            nc.vector.tensor_sub(oh_ap, m0, m1)
            # pad the strip stride to DV+1 so the reduce input AP cannot be
            # collapsed to 2D (the X-axis reduction keys off the innermost dim)
            sq = comb.tile([128, NT, DV + 1], F32, tag="sq", name=f"sq_{h}")
            nc.vector.tensor_mul(sq[:, :, 0:DV], oh_ap, oh_ap)
            ssq_r = ssq_all[:].rearrange("p (h t) -> p h t", h=HPG)
            nc.vector.tensor_reduce(
                out=ssq_r[:, h, :],
                in_=sq[:, :, 0:DV],
                axis=mybir.AxisListType.X,
                op=AluOp.add,
            )
            if h == 3:
                # heads 0-3: RMS scale overlaps heads 4-7's attention
                emit_rms_half(0, 4, defer_scales=True)

        emit_rms_half(4, HPG, defer_scales=False)

    # ---------------- phase 5: output projection ----------------
    with (
        tc.tile_pool(name="pps", bufs=4, space="PSUM") as pps,
        tc.tile_pool(name="yout", bufs=3) as yout,
    ):
        for tt in range(NT):
            yt = yout.tile([128, C], F32, tag="yt", name=f"y{tt}")
            for nh in range(2):
                ps = pps.tile([128, 512], F32, tag="pp", name=f"pp{tt}{nh}")
                for rr in range(4):
                    nc.tensor.matmul(
                        ps,
                        outcatT_sb[:, rr, tt * 128 : (tt + 1) * 128],
                        wp_sb[:, rr, nh * 512 : (nh + 1) * 512],
                        start=(rr == 0),
                        stop=(rr == 3),
                    )
                nc.vector.tensor_copy(out=yt[:, nh * 512 : (nh + 1) * 512], in_=ps)
            # y stores go out on the second HWDGE queue so they don't
            # serialize with the transposes on the sync queue
            nc.scalar.dma_start(out=y[tt * 128 : (tt + 1) * 128, :], in_=yt)


def build_nc():
    nc = bass.Bass()
    xT = nc.declare_dram_parameter("xT", [C, T], F16, isOutput=False)
    w_qk = nc.declare_dram_parameter("w_qk", [C, COLS], F16, isOutput=False)
    w_v = nc.declare_dram_parameter("w_v", [C, 512], F16, isOutput=False)
    w_p = nc.declare_dram_parameter("w_p", [512, C], F16, isOutput=False)
    lam = nc.declare_dram_parameter("lam", [128, 1], F32, isOutput=False)
    y = nc.declare_dram_parameter("y", [T, C], F32, isOutput=True)
    with tile.TileContext(nc) as tc:
        with ExitStack() as ctx:
            _emit(ctx, tc, xT, w_qk, w_v, w_p, lam, y)
    return nc


_NC = None


def _get_nc():
    global _NC
    if _NC is None:
        _NC = build_nc()
    return _NC


def make_in_maps(x, w_attn, w_proj, lambda_q1, lambda_q2, lambda_k1, lambda_k2, gamma):
    x = np.asarray(x, np.float32)
    w_attn = np.asarray(w_attn, np.float32)
    w_proj = np.asarray(w_proj, np.float32)
    lam1 = np.exp(np.sum(np.float32(lambda_q1) * np.float32(lambda_k1), dtype=np.float32))
    lam2 = np.exp(np.sum(np.float32(lambda_q2) * np.float32(lambda_k2), dtype=np.float32))
    lam_full = np.float32(lam1 - lam2 + LAMBDA_INIT)
    lam_tile = np.full((128, 1), lam_full, np.float32)
    # fold gamma * (1 - lambda_init) into w_proj rows
    scale = np.tile(np.asarray(gamma, np.float32), H_TOT) * np.float32(1.0 - LAMBDA_INIT)
    w_p_full = (w_proj * scale[:, None]).astype(np.float16)

    in_maps = []
    for core in range(N_CORES):
        b, g = core // G, core % G
        in_maps.append(
            {
                "xT": np.ascontiguousarray(x[b].T.astype(np.float16)),
                "w_qk": np.ascontiguousarray(
                    np.concatenate(
                        [
                            w_attn[:, g * 512 : (g + 1) * 512],
                            w_attn[:, C + g * 512 : C + (g + 1) * 512],
                        ],
                        axis=1,
                    ).astype(np.float16)
                ),
                "w_v": np.ascontiguousarray(
                    w_attn[:, 2 * C + g * 512 : 2 * C + (g + 1) * 512].astype(
                        np.float16
                    )
                ),
                "w_p": np.ascontiguousarray(w_p_full[g * 512 : (g + 1) * 512, :]),
                "lam": lam_tile,
            }
        )
    return in_maps


def assemble(results):
    y = np.empty((B, T, C), np.float32)
    for b in range(B):
        y[b] = results[b * G]["y"] + results[b * G + 1]["y"]
    return y


def kernel(**inputs) -> np.ndarray:
    nc = _get_nc()
    in_maps = make_in_maps(**inputs)
    res = run_bass_kernel_spmd(nc, in_maps, list(range(N_CORES)))
    return assemble(res.results)
